# revision 18
# baseline (speedup 1.0000x reference)
"""Trainium2 Bass kernel for nn_InterfaceGraph (retrieval_knn).

Segment-restricted nearest neighbors between pos_a and pos_b (16384 x
16384 pairwise distances, block-diagonal over 64 sorted graphs), sharded
over 8 NeuronCores.

Geometry (per side, independent): graphs are LPT-balanced across cores
by row count and slot-sorted by row count desc.  Every slot gets 2
regular 128-row tiles (rows 0..256) sharing ONE rhs segment; the
leftover rows (256..na <= 64 of them) of big slots are packed two
slots per extra "remainder" tile at row offsets 0/64, each member with
its own small rhs segment and an accumulating second matmul (a
member's lhs columns are zero outside its rows, so the overlay is
exact: 0 + x == x in fp32).  This keeps rhs bytes near-minimal (input
DMA is startup-critical) while cutting the near-empty third tiles of
the naive per-graph tiling.

Per tile, ONE fused custom DVE instruction (ARGMAX_PACK) does the
whole argmax in a single pass over PSUM: per element it clears the low
9 mantissa bits of the fp32 key (key = 2 a.b - |b|^2, bf16x3-split
K=21 matmul) and ORs in the column index from a constant iota-bits
tensor; a MAX accumulator folds the packed values to [128,1] per tile.
Float-max then orders keys at 2^-14 relative-to-|key| quantization
with the index riding in the low bits: argmin index = accum & 0x1FF.

Host does the O(N) epilogue: gather + norm, an exact-recompute band
near the 10.0 interface cutoff (covers the quantization), residue
segment-max mask, mutation OR.
"""

import numpy as np
import ml_dtypes

NCORES = 8
G = 64
GPC = G // NCORES
NUM_RESIDUES = 2048
CUTOFF = np.float32(10.0)
BIG = np.float32(2.0 ** 26)
K = 21            # 9 tier-2 + 6 tier-1 + 3 tier-0 cross rows + 3 |b|^2 rows
WMAX = 512        # PSUM bank width; index must fit the low 9 bits
REGT = 2          # regular tiles per slot (rows 0..256)
REMCAP = 64       # remainder rows per member slot (row offsets 0/64)

PROFILE = False
LAST_EXEC_NS = None

BF16 = ml_dtypes.bfloat16
FLT_MAX = np.float32(3.4028235e38)
IDX_MASK = np.uint32(0x1FF)
MASK_CONST = float(np.uint32(0x1FF).view(np.float32))  # denormal, bits 0x1FF

_prog_cache = {}
_argmax_op = None


def _round_up(x, m):
    return (x + m - 1) // m * m


def _install_ntff_hook():
    import sys
    import types
    if 'antenv.axon_hooks' in sys.modules:
        return
    from trn_agent_boot.trn_boot import _ntff_profile_via_ctypes
    hook = _ntff_profile_via_ctypes('/opt/axon/libaxon_pjrt.so')
    mod = types.ModuleType('antenv.axon_hooks')
    mod.get_axon_ntff_profile_hook = lambda: hook
    sys.modules['antenv.axon_hooks'] = mod


def _get_argmax_op():
    """Register the one-pass packed-argmax custom DVE op.

    body = (Src0 ^ (Src0 & C0)) | Src1  with C0 = bits 0x1FF (a denormal)
    == (key & ~0x1FF) | iota_bits, folded with a MAX accumulator.
    For the distance keys, float-max over the packed values picks the
    quantized-max key; ties take one index inside the quantum.
    """
    global _argmax_op
    if _argmax_op is not None:
        return _argmax_op

    import concourse.dve_ops as dve_ops
    from concourse.dve_ops import DveOp
    from concourse.dve_spec import (
        Spec, Src0, Src1, Bin, lower, _has_src1, AluOp, C0, maxx)
    from concourse.dve_uop import DveOpSpec

    name = "ARGMAX_PACK_F32_ANT"
    if name in dve_ops._SUB_OPCODE_FOR_NAME:
        _argmax_op = next(o for o in dve_ops.OPS if o.name == name)
        return _argmax_op

    def _ref(in0, in1, s0, s1, imm2):
        m = ~np.float32(s0).view(np.uint32)
        x = np.ascontiguousarray(in0.astype(np.float32)).view(np.uint32)
        i = np.ascontiguousarray(in1.astype(np.float32)).view(np.uint32)
        b = ((x & m) | i).view(np.float32)
        acc = np.maximum(
            b.reshape(b.shape[0], -1).max(axis=1, keepdims=True), -FLT_MAX)
        return b, acc

    low = Bin(AluOp.BITWISE_AND, Src0, C0)
    spec = Spec(
        body=Bin(AluOp.BITWISE_OR, Bin(AluOp.BITWISE_XOR, Src0, low), Src1),
        accum=maxx, reference=_ref)

    row = dve_ops._CUSTOM_DVE_ROW_BASE + len(dve_ops.OPS)
    assert row < 0x20
    shas = {}
    for ver in ("v3", "v4"):
        s = DveOpSpec(name=name, opcode=row, uops=lower(spec, ver=ver),
                      rd1_en=_has_src1(spec))
        shas[ver] = s.sha(ver)
    op = DveOp(name, spec, subdim=False, uops_sha=shas)
    dve_ops.OPS.append(op)
    dve_ops.CUSTOM_DVE_SPECS[name] = spec
    dve_ops._SUB_OPCODE_FOR_NAME[name] = row
    _argmax_op = op
    return op


def _split3(v):
    """bf16x3 split: v ~= v1 + v2 + v3 with ~24-bit mantissa coverage."""
    v = v.astype(np.float32)
    v1 = v.astype(BF16).astype(np.float32)
    r = v - v1
    v2 = r.astype(BF16).astype(np.float32)
    v3 = (r - v2).astype(BF16).astype(np.float32)
    return v1, v2, v3


class _SidePlan:
    """Slot + remainder tiling for one direction."""

    def __init__(self, row_sizes, col_sizes):
        order = np.argsort(-row_sizes, kind="stable")
        loads = np.zeros(NCORES, dtype=np.int64)
        members = [[] for _ in range(NCORES)]
        for g in order:
            c = int(np.argmin(loads))
            loads[c] += int(row_sizes[g])
            members[c].append(int(g))
        for c in range(NCORES):
            members[c].sort(key=lambda g: (-int(row_sizes[g]), g))
        self.members = members            # [core][slot] -> graph
        maxr = [max(int(row_sizes[members[c][j]]) for c in range(NCORES))
                for j in range(GPC)]
        assert all(128 < r <= REGT * 128 + REMCAP for r in maxr)
        self.slotW = [max(8, _round_up(
            max(int(col_sizes[members[c][j]]) for c in range(NCORES)), 4))
            for j in range(GPC)]
        assert max(self.slotW) <= WMAX
        self.rem_slots = [j for j in range(GPC) if maxr[j] > REGT * 128]
        self.pairs = [tuple(self.rem_slots[i:i + 2])
                      for i in range(0, len(self.rem_slots), 2)]
        self.remW = [max(8, _round_up(max(self.slotW[j] for j in p), 4))
                     for p in self.pairs]
        self.T = GPC * REGT + len(self.pairs)
        # lhs columns: regular (j, t) at (REGT*j + t)*128; remainder pair
        # p member m at (GPC*REGT + 2p + m)*128
        self.L = (GPC * REGT + 2 * len(self.pairs)) * 128
        # rhs columns: slot segments then per-pair member segments
        self.sbase = []
        off = 0
        for j in range(GPC):
            self.sbase.append(off)
            off += self.slotW[j]
        self.rbase = []
        for p, pr in enumerate(self.pairs):
            self.rbase.append(off)
            off += 2 * self.remW[p]
        self.R = off
        # chunk splits: slots 0-1 land first so compute starts ASAP,
        # slots 2-7 second, remainder segments last
        self.lsplit1 = REGT * 2 * 128
        self.lsplit2 = GPC * REGT * 128
        self.rsplit1 = self.sbase[2]
        self.rsplit2 = self.rbase[0] if self.pairs else self.R

    def key(self):
        return (tuple(self.slotW), tuple(self.pairs), tuple(self.remW))


def _build_program(pa, pb, wm):
    from contextlib import ExitStack

    import concourse.bacc as bacc
    import concourse.mybir as mybir
    import concourse.tile as tile

    f32 = mybir.dt.float32
    bf16 = mybir.dt.bfloat16

    op_argmax = _get_argmax_op()

    nc = bacc.Bacc("TRN2", target_bir_lowering=False, debug=False,
                   enable_asserts=True, num_devices=NCORES)

    iota = nc.dram_tensor("iota", [128, wm], f32, kind="ExternalInput").ap()

    def chunk_widths(pl):
        return {"l1": pl.lsplit1, "l2": pl.lsplit2 - pl.lsplit1,
                "lr": pl.L - pl.lsplit2,
                "r1": pl.rsplit1, "r2": pl.rsplit2 - pl.rsplit1,
                "rr": pl.R - pl.rsplit2}

    dr = {}
    for sd, pl in (("A", pa), ("B", pb)):
        for ck, n in chunk_widths(pl).items():
            if n > 0:
                nm = f"{ck}{sd}"
                dr[nm] = nc.dram_tensor(
                    nm, [K, n], bf16, kind="ExternalInput").ap()
    acc = nc.dram_tensor("acc", [128, pa.T + pb.T], f32,
                         kind="ExternalOutput").ap()

    with tile.TileContext(nc) as tc:
        with ExitStack() as ctx:
            const = ctx.enter_context(tc.tile_pool(name="const", bufs=1))
            psum = ctx.enter_context(
                tc.tile_pool(name="psum", bufs=8, space="PSUM"))

            # PE warmup first: dummy matmuls over a memzeroed scratch (no
            # DMA dependency, so they start right after the preamble) keep
            # the tensor engine busy through the input-DMA window; its
            # DVFS ramp is then warm when the real tiles arrive.  Results
            # are garbage and never read.
            warm = const.tile([K, 256], bf16, tag="warm")
            nc.gpsimd.memset(warm[:], 0.0)
            for _ in range(16):
                wps = psum.tile([128, 128], f32, tag="ps")
                nc.tensor.matmul(wps[:], warm[:, 0:128], warm[:, 128:256],
                                 start=True, stop=True)

            # iota gates the first argmax; input chunks spread over the
            # three DMA-capable sequencers, first-needed chunks on
            # separate issuers so slot-0/1 data lands ~2us after the
            # preamble
            iota_sb = const.tile([128, wm], f32, tag="iota")
            nc.sync.dma_start(iota_sb[:], iota[:])

            issue = {"l1A": nc.sync, "r1A": nc.scalar,
                     "l2A": nc.gpsimd, "r2A": nc.gpsimd,
                     "l1B": nc.scalar, "r1B": nc.scalar,
                     "l2B": nc.gpsimd, "r2B": nc.gpsimd,
                     "lrA": nc.sync, "rrA": nc.sync,
                     "lrB": nc.sync, "rrB": nc.sync}
            order = ("l1A", "r1A", "l2A", "r2A", "lrA", "rrA",
                     "l1B", "r1B", "l2B", "r2B", "lrB", "rrB")
            sb = {}
            for nm in order:
                if nm in dr:
                    t = const.tile(list(dr[nm].shape), bf16, tag=nm)
                    issue[nm].dma_start(t[:], dr[nm][:])
                    sb[nm] = t

            acc_sb = const.tile([128, pa.T + pb.T], f32, tag="acc")

            def argmax(ps, w, col):
                nc.vector._custom_dve(
                    op_argmax, out=ps[:],
                    accum_out=acc_sb[:, col:col + 1],
                    in0=ps[:], in1=iota_sb[:, :w], s0=MASK_CONST)

            def side(pl, sd, acc_off):
                def lhs_ap(col0):
                    if col0 < pl.lsplit1:
                        return sb[f"l1{sd}"][:, col0:col0 + 128]
                    if col0 < pl.lsplit2:
                        c = col0 - pl.lsplit1
                        return sb[f"l2{sd}"][:, c:c + 128]
                    c = col0 - pl.lsplit2
                    return sb[f"lr{sd}"][:, c:c + 128]

                def rhs_ap(col0, w):
                    if col0 < pl.rsplit1:
                        return sb[f"r1{sd}"][:, col0:col0 + w]
                    if col0 < pl.rsplit2:
                        c = col0 - pl.rsplit1
                        return sb[f"r2{sd}"][:, c:c + w]
                    c = col0 - pl.rsplit2
                    return sb[f"rr{sd}"][:, c:c + w]

                for j in range(GPC):
                    w = pl.slotW[j]
                    for t in range(REGT):
                        ps = psum.tile([128, w], f32, tag="ps")
                        nc.tensor.matmul(
                            ps[:], lhs_ap((REGT * j + t) * 128),
                            rhs_ap(pl.sbase[j], w),
                            start=True, stop=True)
                        argmax(ps, w, acc_off + REGT * j + t)
                for p, pr in enumerate(pl.pairs):
                    w = pl.remW[p]
                    ps = psum.tile([128, w], f32, tag="ps")
                    for m, j in enumerate(pr):
                        nc.tensor.matmul(
                            ps[:],
                            lhs_ap((GPC * REGT + 2 * p + m) * 128),
                            rhs_ap(pl.rbase[p] + m * w, w),
                            start=(m == 0), stop=(m == len(pr) - 1))
                    argmax(ps, w, acc_off + GPC * REGT + p)

            side(pa, "A", 0)
            # A-side results stream out while the B side computes
            nc.sync.dma_start(acc[:, :pa.T], acc_sb[:, :pa.T])
            side(pb, "B", pa.T)
            nc.sync.dma_start(acc[:, pa.T:], acc_sb[:, pa.T:])

    nc.compile()
    return nc


def _lhs_block(pos_rows):
    """[K, n] stationary pack for n row atoms: bf16x3 of 2*p plus the
    -1 rows (9/16/20) that apply the moving side's -|q|^2 / -BIG."""
    n = pos_rows.shape[0]
    blk = np.zeros((K, n), dtype=np.float32)
    blk[9, :] = -1.0
    blk[16, :] = -1.0
    blk[20, :] = -1.0
    for c in range(3):
        a1, a2, a3 = _split3(np.float32(2.0) * pos_rows[:, c])
        blk[0 + c * 3] = a1
        blk[1 + c * 3] = a2
        blk[2 + c * 3] = a3
        blk[10 + c * 2] = a1
        blk[11 + c * 2] = a2
        blk[17 + c] = a1
    return blk


def _rhs_block(pos_cols):
    """[K, m] moving pack for m column atoms: bf16x3 coords + |q|^2."""
    m = pos_cols.shape[0]
    blk = np.zeros((K, m), dtype=np.float32)
    q = pos_cols
    qq = (q[:, 0] * q[:, 0] + q[:, 1] * q[:, 1]) + q[:, 2] * q[:, 2]
    q1, q2, q3 = _split3(qq)
    for c in range(3):
        b1, b2, b3 = _split3(q[:, c])
        blk[0 + c * 3] = b3
        blk[1 + c * 3] = b2
        blk[2 + c * 3] = b1
        blk[10 + c * 2] = b2
        blk[11 + c * 2] = b1
        blk[17 + c] = b1
    blk[9] = q3
    blk[16] = q2
    blk[20] = q1
    return blk


def _pack_side(pl, c, row_sizes, starts_row, starts_col,
               lblocks, rblocks, lhs, rhs):
    """Fill one core's fp32 staging lhs [K, pl.L] / rhs [K, pl.R]."""
    for j in range(GPC):
        g = pl.members[c][j]
        n = int(row_sizes[g])
        w = pl.slotW[j]
        lb = REGT * j * 128
        take = min(n, REGT * 128)
        lhs[:, lb:lb + take] = lblocks[g][:, :take]
        rb = pl.sbase[j]
        nbg = rblocks[g].shape[1]
        rhs[:, rb:rb + nbg] = rblocks[g]
        rhs[20, rb + nbg:rb + w] = BIG
    for p, pr in enumerate(pl.pairs):
        w = pl.remW[p]
        for m, j in enumerate(pr):
            g = pl.members[c][j]
            n = int(row_sizes[g])
            rem = n - REGT * 128
            # member m's lhs segment; its rows sit at tile offset m*REMCAP
            lb = (GPC * REGT + 2 * p + m) * 128
            if rem > 0:
                lhs[:, lb + m * REMCAP:lb + m * REMCAP + rem] = \
                    lblocks[g][:, REGT * 128:REGT * 128 + rem]
            rb = pl.rbase[p] + m * w
            nbg = rblocks[g].shape[1]
            rhs[:, rb:rb + nbg] = rblocks[g]
            rhs[20, rb + nbg:rb + w] = BIG


def _unpack_side(pl, c, acc, row_sizes, starts_row, starts_col, idx_full):
    bits = np.ascontiguousarray(acc.astype(np.float32)).view(np.uint32)
    for j in range(GPC):
        g = pl.members[c][j]
        n = int(row_sizes[g])
        for t in range(REGT):
            rows = min(128, n - t * 128)
            if rows <= 0:
                break
            jcol = (bits[:rows, REGT * j + t] & IDX_MASK).astype(np.int64)
            atoms = starts_row[g] + t * 128 + np.arange(rows)
            idx_full[atoms] = starts_col[g] + jcol
    for p, pr in enumerate(pl.pairs):
        for m, j in enumerate(pr):
            g = pl.members[c][j]
            rem = int(row_sizes[g]) - REGT * 128
            if rem <= 0:
                continue
            r0 = m * REMCAP
            jcol = (bits[r0:r0 + rem, GPC * REGT + p]
                    & IDX_MASK).astype(np.int64)
            atoms = starts_row[g] + REGT * 128 + np.arange(rem)
            idx_full[atoms] = starts_col[g] + jcol


def _fix_band(pos_row, pos_col, n2g_row, starts_col, dist, idx):
    """Exact recompute for rows whose dist lands near the 10.0 cutoff.

    The packed argmax quantizes keys at 2^-14 of |key| (|key| can reach
    |a|^2 ~ 1e4, i.e. ~0.7 absolute in d^2), so a selected neighbor can
    sit a few hundredths above the true min dist near the threshold.
    Redo a generous band with the reference formula.
    """
    band = np.nonzero(np.abs(dist - CUTOFF) < np.float32(0.15))[0]
    for atom in band:
        g = int(n2g_row[atom])
        seg = pos_col[starts_col[g]:starts_col[g + 1]]
        p = pos_row[atom]
        d2 = ((p * p).sum() + (seg * seg).sum(axis=1)
              - np.float32(2.0) * (seg @ p))
        j = int(np.argmin(d2))
        idx[atom] = starts_col[g] + j
        da = p - seg[j]
        dist[atom] = np.sqrt((da[0] * da[0] + da[1] * da[1]) + da[2] * da[2])


def kernel(pos_a, pos_b, node2graph_a, node2graph_b,
           atom2residue_a, atom2residue_b, is_mutation):
    global LAST_EXEC_NS

    from concourse.bass_utils import run_bass_kernel_spmd

    pos_a = np.asarray(pos_a, dtype=np.float32)
    pos_b = np.asarray(pos_b, dtype=np.float32)
    node2graph_a = np.asarray(node2graph_a)
    node2graph_b = np.asarray(node2graph_b)
    atom2residue_a = np.asarray(atom2residue_a)
    atom2residue_b = np.asarray(atom2residue_b)
    is_mutation = np.asarray(is_mutation)

    Na = pos_a.shape[0]
    Nb = pos_b.shape[0]

    sa = np.searchsorted(node2graph_a, np.arange(G + 1)).astype(np.int64)
    sb = np.searchsorted(node2graph_b, np.arange(G + 1)).astype(np.int64)
    na = np.diff(sa)
    nb = np.diff(sb)
    assert na.min() > 0 and nb.min() > 0, "empty graph block not supported"

    pa = _SidePlan(na, nb)    # rows = a atoms, cols = b atoms
    pb = _SidePlan(nb, na)    # rows = b atoms, cols = a atoms
    wm = max(max(pa.slotW), max(pa.remW, default=8),
             max(pb.slotW), max(pb.remW, default=8))

    key = (pa.key(), pb.key(), wm)
    if key not in _prog_cache:
        _prog_cache[key] = _build_program(pa, pb, wm)
    nc = _prog_cache[key]

    iota_bits = np.arange(wm, dtype=np.uint32)[None, :].repeat(
        128, axis=0).view(np.float32)

    lblkA = {g: _lhs_block(pos_a[sa[g]:sa[g + 1]]) for g in range(G)}
    rblkB = {g: _rhs_block(pos_b[sb[g]:sb[g + 1]]) for g in range(G)}
    lblkB = {g: _lhs_block(pos_b[sb[g]:sb[g + 1]]) for g in range(G)}
    rblkA = {g: _rhs_block(pos_a[sa[g]:sa[g + 1]]) for g in range(G)}

    in_maps = []
    for c in range(NCORES):
        lhsA = np.zeros((K, pa.L), dtype=np.float32)
        rhsB = np.zeros((K, pa.R), dtype=np.float32)
        _pack_side(pa, c, na, sa, sb, lblkA, rblkB, lhsA, rhsB)
        lhsB = np.zeros((K, pb.L), dtype=np.float32)
        rhsA = np.zeros((K, pb.R), dtype=np.float32)
        _pack_side(pb, c, nb, sb, sa, lblkB, rblkA, lhsB, rhsA)
        m = {"iota": iota_bits}
        for sd, pl, lhs, rhs in (("A", pa, lhsA, rhsB),
                                 ("B", pb, lhsB, rhsA)):
            parts = {
                f"l1{sd}": lhs[:, :pl.lsplit1],
                f"l2{sd}": lhs[:, pl.lsplit1:pl.lsplit2],
                f"lr{sd}": lhs[:, pl.lsplit2:],
                f"r1{sd}": rhs[:, :pl.rsplit1],
                f"r2{sd}": rhs[:, pl.rsplit1:pl.rsplit2],
                f"rr{sd}": rhs[:, pl.rsplit2:],
            }
            for nm, arr in parts.items():
                if arr.shape[1] > 0:
                    m[nm] = arr.astype(BF16)
        in_maps.append(m)

    if PROFILE:
        _install_ntff_hook()
    res = run_bass_kernel_spmd(nc, in_maps, list(range(NCORES)),
                               trace=bool(PROFILE))
    if PROFILE:
        LAST_EXEC_NS = res.exec_time_ns

    idx_a = np.zeros(Na, dtype=np.int64)
    idx_b = np.zeros(Nb, dtype=np.int64)
    for c in range(NCORES):
        acc = res.results[c]["acc"]
        _unpack_side(pa, c, acc[:, :pa.T], na, sa, sb, idx_a)
        _unpack_side(pb, c, acc[:, pa.T:], nb, sb, sa, idx_b)

    da = pos_a - pos_b[idx_a]
    dist_a = np.sqrt((da[:, 0] * da[:, 0] + da[:, 1] * da[:, 1])
                     + da[:, 2] * da[:, 2])
    db = pos_b - pos_a[idx_b]
    dist_b = np.sqrt((db[:, 0] * db[:, 0] + db[:, 1] * db[:, 1])
                     + db[:, 2] * db[:, 2])

    _fix_band(pos_a, pos_b, node2graph_a, sb, dist_a, idx_a)
    _fix_band(pos_b, pos_a, node2graph_b, sa, dist_b, idx_b)

    def iface_mask(dist, atom2residue):
        is_if = (dist < CUTOFF).astype(np.int32)
        res_max = np.zeros(NUM_RESIDUES, dtype=np.int32)
        np.maximum.at(res_max, atom2residue, is_if)
        return res_max[atom2residue] > 0

    mask_a = iface_mask(dist_a, atom2residue_a)
    mask_b = iface_mask(dist_b, atom2residue_b)
    mask = np.concatenate([mask_a, mask_b]) | is_mutation.astype(bool)
    dists = np.concatenate([dist_a, dist_b]).astype(np.float32)
    return mask, dists
